# revision 1
# baseline (speedup 1.0000x reference)
"""Distributed Trainium2 kernel for nn_Attention_64854006169830.

Strategy (8 NeuronCores, SPMD):
  - QKV + attention: head-parallel (core i computes head i for all B*L rows),
    with activations kept feature-major ("transposed") so every matmul uses
    natural weight layouts. Softmax is computed on transposed scores
    (keys on partitions): exp on ACT, row-sums via ones-matmul on PE,
    normalization after PE-transpose back to row-major.
  - ctx redistribution head-shard -> row-shard via per-batch AllToAll
    (overlapped with attention of later batches).
  - LN + FF + collapse(d->1): row-parallel (core i owns 256 L-positions of
    each batch). Softmax denominators use a depth-2 bf16 add-tree on DVE
    plus a few ones-matmuls on PE.
  - c = [B, L] gathered with a tiny AllGather; the final two small matmuls
    (L->L, L->OUT) are column/row-sharded with a closing 4KB AllReduce.
Compute dtype: bf16 (f32 accumulation in PSUM); ~0.55% rel err vs the
float32 reference (gate is 2e-2). HW exec time ~520-545us on 8 cores.
"""
import sys
import math

for _p in ("/opt/trn_rl_repo", "/opt/trn_rl_repo/concourse"):
    if _p not in sys.path:
        sys.path.insert(0, _p)

import numpy as np
import ml_dtypes

B, L, D, H, OUT = 4, 2048, 1024, 8, 256
DH = D // H          # 128
N = B * L            # 8192 rows
NC = 8               # cores
RPC = N // NC        # 1024 rows per core (as 4 batches x 256 L-positions)
LPC = L // NC        # 256 L-positions per core per batch
EPS = 1e-12

_CACHE = {}


def _build_nc(trivial_gb=False):
    import concourse.bass as bass
    import concourse.tile as tile
    from concourse import bacc, mybir
    from concourse.masks import make_identity

    BF = mybir.dt.bfloat16
    F32 = mybir.dt.float32
    AF = mybir.ActivationFunctionType
    OP = mybir.AluOpType

    nc = bacc.Bacc("TRN2", debug=False, num_devices=NC)

    # ---- parameters (per-core values supplied via in_maps) ----
    xT = nc.dram_tensor("xT", [D, N], BF, kind="ExternalInput")
    xrows = nc.dram_tensor("xrows", [RPC, D], BF, kind="ExternalInput")
    wqkv = nc.dram_tensor("wqkv", [3, D, DH], BF, kind="ExternalInput")
    bqkv = nc.dram_tensor("bqkv", [3, DH], F32, kind="ExternalInput")
    wff = nc.dram_tensor("wff", [D, D], BF, kind="ExternalInput")
    bff = nc.dram_tensor("bff", [D], BF, kind="ExternalInput")
    gamma = nc.dram_tensor("gamma", [D], BF, kind="ExternalInput")
    beta = nc.dram_tensor("beta", [D], BF, kind="ExternalInput")
    wc1 = nc.dram_tensor("wc1", [D, D], BF, kind="ExternalInput")   # gamma-folded
    bc1 = nc.dram_tensor("bc1", [D], F32, kind="ExternalInput")     # beta-folded
    wc2 = nc.dram_tensor("wc2", [D], BF, kind="ExternalInput")
    bc2 = nc.dram_tensor("bc2", [1], F32, kind="ExternalInput")
    wl1s = nc.dram_tensor("wl1s", [L, L // NC], BF, kind="ExternalInput")
    bl1s = nc.dram_tensor("bl1s", [L // NC], BF, kind="ExternalInput")
    wl2s = nc.dram_tensor("wl2s", [L // NC, OUT], BF, kind="ExternalInput")
    bl2 = nc.dram_tensor("bl2", [OUT], F32, kind="ExternalInput")
    out = nc.dram_tensor("out", [B, OUT], F32, kind="ExternalOutput")

    # ---- internal DRAM ----
    a2a_in = nc.dram_tensor("a2a_in", [N, DH], BF)
    a2a_out = nc.dram_tensor("a2a_out", [N, DH], BF)
    sums_hbm = nc.dram_tensor("sums_hbm", [N], F32)
    c_ag_in = nc.dram_tensor("c_ag_in", [RPC], BF)
    c_ag_out = nc.dram_tensor("c_ag_out", [N], BF, addr_space="Shared")
    ar_in = nc.dram_tensor("ar_in", [B, OUT], F32)
    ar_out = nc.dram_tensor("ar_out", [B, OUT], F32, addr_space="Shared")

    def bcast(dram_handle, parts, free):
        """Broadcast a [free] DRAM vector across `parts` partitions."""
        ap = dram_handle.ap()
        return bass.AP(tensor=ap.tensor, offset=0, ap=[[0, parts], [1, free]])

    RG = [list(range(NC))]
    ISQ = 1.0  # 1/sqrt(DH) folded into wq/bq on host

    from contextlib import ExitStack

    with tile.TileContext(nc) as tc, ExitStack() as root:
        glob = root.enter_context(tc.tile_pool(name="glob", bufs=1))
        ident = glob.tile([128, 128], BF)
        make_identity(nc, ident[:])
        ones128 = glob.tile([128, 1], BF)
        nc.vector.memset(ones128[:], 1.0)
        eps_sb = glob.tile([128, 1], F32)
        nc.vector.memset(eps_sb[:], EPS)

        # Phase-C weight pool carved out first so its loads never overlap
        # (in address space) with the big transient phase-A/B tiles.
        wC_pool = root.enter_context(tc.tile_pool(name="wC", bufs=1))

        phAB = root.enter_context(ExitStack())
        qkv_pool = phAB.enter_context(tc.tile_pool(name="qkv", bufs=1))
        # persistent through phases A+B; per-batch tiles so attention on
        # batch b can start as soon as batch b's QKV is done
        qkvT = [qkv_pool.tile([128, 2, L], BF, name=f"qkvT{b}") for b in range(B)]
        vnat = [qkv_pool.tile([128, L // 128, DH], BF, name=f"vnat{b}")
                for b in range(B)]

        # ================= Phase A: QKV^T =================
        with ExitStack() as phA:
            xt_pool = phA.enter_context(tc.tile_pool(name="xt", bufs=2))
            wq_pool = phA.enter_context(tc.tile_pool(name="wqkv", bufs=1))
            psA = phA.enter_context(tc.tile_pool(name="psA", bufs=6, space="PSUM"))
            psTrA = phA.enter_context(tc.tile_pool(name="psTrA", bufs=2, space="PSUM"))
            vstage_pool = phA.enter_context(tc.tile_pool(name="vstage", bufs=2))

            # weights first (tiny) so the first matmuls aren't stuck behind
            # the 16MB x^T load in the DMA queues
            wq_sb = wq_pool.tile([128, 3, D // 128, DH], BF)
            nc.sync.dma_start(
                out=wq_sb[:],
                in_=bass.AP(tensor=wqkv.ap().tensor, offset=0,
                            ap=[[DH, 128], [D * DH, 3], [128 * DH, D // 128], [1, DH]]))
            bq_sb = wq_pool.tile([128, 3], F32)
            nc.sync.dma_start(
                out=bq_sb[:],
                in_=bass.AP(tensor=bqkv.ap().tensor, offset=0,
                            ap=[[1, 128], [DH, 3]]))

            # row-group-major so compute on group g starts right after its DMA
            for rg in range(4):
                xt = xt_pool.tile([128, D // 128, L], BF, tag="xt",
                                  name=f"xt{rg}")
                for kc in range(D // 128):
                    xt_last_dma = nc.sync.dma_start(
                        out=xt[:, kc, :],
                        in_=xT.ap()[kc * 128:(kc + 1) * 128,
                                    rg * 2048:(rg + 1) * 2048])
                for s in range(3):
                    pst = [psA.tile([128, 512], F32, tag="qkvps", name=f"qkvps{rg}_{s}_{j}")
                           for j in range(4)]
                    for kc in range(D // 128):
                        for r4 in range(4):
                            rc = rg * 4 + r4
                            nc.tensor.matmul(
                                pst[r4][:], wq_sb[:, s, kc, :],
                                xt[:, kc, r4 * 512:(r4 + 1) * 512],
                                start=(kc == 0), stop=(kc == D // 128 - 1))
                    for r4 in range(4):
                        rc = rg * 4 + r4
                        if s < 2:
                            nc.vector.tensor_scalar_add(
                                qkvT[rg][:, s, r4 * 512:(r4 + 1) * 512], pst[r4][:],
                                bq_sb[:, s:s + 1])
                        else:
                            # v: bias-add to staging, then PE-transpose into
                            # row-major vnat
                            vstage = vstage_pool.tile([128, 512], BF, tag="vstage",
                                                      name=f"vst{rg}_{r4}")
                            nc.vector.tensor_scalar_add(
                                vstage[:], pst[r4][:], bq_sb[:, s:s + 1])
                            tps = psTrA.tile([128, 4, 128], BF, tag="vtr",
                                             name=f"vtr{rg}_{r4}")
                            for j in range(4):
                                nc.tensor.transpose(
                                    tps[:, j, :], vstage[:, j * 128:(j + 1) * 128],
                                    ident[:])
                            nc.vector.tensor_copy(
                                vnat[rg][:, r4 * 4:(r4 + 1) * 4, :], tps[:])

        # ================= Phase B: attention per batch =================
        with ExitStack() as phB:
            pT_pool = phB.enter_context(tc.tile_pool(name="pT", bufs=2))
            red_pool = phB.enter_context(tc.tile_pool(name="red", bufs=1))
            ctxT_pool = phB.enter_context(tc.tile_pool(name="ctxT", bufs=2))
            sums_pool = phB.enter_context(tc.tile_pool(name="sums", bufs=1))
            recip_pool = phB.enter_context(tc.tile_pool(name="recip", bufs=2))
            norm_pool = phB.enter_context(tc.tile_pool(name="norm", bufs=3))
            psS = phB.enter_context(tc.tile_pool(name="psS", bufs=2, space="PSUM"))
            psC = phB.enter_context(tc.tile_pool(name="psC", bufs=2, space="PSUM"))
            psSum = phB.enter_context(tc.tile_pool(name="psSum", bufs=1, space="PSUM"))
            psTrB = phB.enter_context(tc.tile_pool(name="psTrB", bufs=1, space="PSUM"))

            KCB = L // 128  # 16 key chunks per batch

            a2a_insts = []
            for b in range(B):
                ctxT_sb = ctxT_pool.tile([128, L], BF, tag="ctxT")
                pass
                for qc in range(L // 1024):
                    pT = pT_pool.tile([128, KCB, 1024], BF, tag="pT")
                    sums_sb = sums_pool.tile([1, 1024], F32, tag="sums", bufs=2,
                                             name=f"sums{b}_{qc}")
                    q0 = qc * 1024
                    for kc in range(KCB):
                        sps = psS.tile([128, 1024], F32, tag="sps")
                        for hh in range(2):
                            nc.tensor.matmul(
                                sps[:, hh * 512:(hh + 1) * 512],
                                qkvT[b][:, 1, kc * 128:(kc + 1) * 128],
                                qkvT[b][:, 0, q0 + hh * 512: q0 + (hh + 1) * 512],
                                start=True, stop=True)
                        nc.scalar.activation(pT[:, kc, :], sps[:], AF.Exp, scale=ISQ)
                    cps2 = [psC.tile([128, 512], F32, tag="cps", name=f"cps{b}_{qc}_{h}")
                            for h in range(2)]
                    for kc in range(KCB):
                        for hh in range(2):
                            nc.tensor.matmul(cps2[hh][:], vnat[b][:, kc, :],
                                             pT[:, kc, hh * 512:(hh + 1) * 512],
                                             start=(kc == 0), stop=(kc == KCB - 1))
                    for hh in range(2):
                        hsl = slice(hh * 512, (hh + 1) * 512)
                        cps = cps2[hh]
                        sps2 = psSum.tile([1, 512], F32, tag="sps2")
                        # softmax denominators: depth-2 bf16 add-tree on DVE,
                        # then 4 ones-matmuls (instead of 16) on PE
                        red8 = red_pool.tile([128, 8, 512], BF, tag="red8",
                                             name=f"red8_{b}_{qc}_{hh}")
                        for j in range(8):
                            nc.vector.tensor_add(red8[:, j, :],
                                                 pT[:, 2 * j, hsl],
                                                 pT[:, 2 * j + 1, hsl])
                        for j in range(4):
                            nc.vector.tensor_add(red8[:, j, :],
                                                 red8[:, 2 * j, :],
                                                 red8[:, 2 * j + 1, :])
                        for j in range(4):
                            nc.tensor.matmul(sps2[:], ones128[:], red8[:, j, :],
                                             start=(j == 0), stop=(j == 3))
                        nc.vector.tensor_copy(
                            ctxT_sb[:, qc * 1024 + hh * 512: qc * 1024 + (hh + 1) * 512],
                            cps[:])
                        nc.vector.tensor_copy(sums_sb[:, hh * 512:(hh + 1) * 512],
                                              sps2[:])
                    # per-qc epilogue: recip roundtrip, transpose back to
                    # row-major, normalize, store this 1024-row slice
                    q_hbm = b * L + qc * 1024
                    nc.sync.dma_start(
                        out=sums_hbm.ap()[q_hbm:q_hbm + 1024].rearrange(
                            "(o n) -> o n", o=1),
                        in_=sums_sb[0:1, :])
                    rraw = recip_pool.tile([128, 8], F32, tag="rraw",
                                           name=f"rraw{b}_{qc}")
                    nc.sync.dma_start(
                        out=rraw[:],
                        in_=sums_hbm.ap()[q_hbm:q_hbm + 1024].rearrange(
                            "(j p) -> p j", p=128))
                    rcols = recip_pool.tile([128, 8], F32, tag="rcols",
                                            name=f"rcols{b}_{qc}")
                    nc.vector.reciprocal(rcols[:], rraw[:])
                    nrm = norm_pool.tile([128, 8, DH], BF, tag="nrm",
                                         name=f"nrm{b}_{qc}")
                    tpsq = psTrB.tile([128, 8, 128], BF, tag="ctr",
                                      name=f"ctr{b}_{qc}")
                    for j in range(8):
                        nc.tensor.transpose(
                            tpsq[:, j, :],
                            ctxT_sb[:, qc * 1024 + j * 128: qc * 1024 + (j + 1) * 128],
                            ident[:])
                    for j in range(8):
                        nc.vector.tensor_scalar_mul(nrm[:, j, :], tpsq[:, j, :],
                                                    rcols[:, j:j + 1])
                    nc.sync.dma_start(
                        out=bass.AP(tensor=a2a_in.ap().tensor, offset=q_hbm * DH,
                                    ap=[[DH, 128], [128 * DH, 8], [1, DH]]),
                        in_=nrm[:])
                a2a_insts.append(nc.gpsimd.collective_compute(
                    "AllToAll", OP.bypass,
                    ins=[a2a_in.ap()[b * L:(b + 1) * L, :]],
                    outs=[a2a_out.ap()[b * L:(b + 1) * L, :]],
                    replica_groups=RG))

        phAB.close()  # release qkvT/vnat space for phase C

        # ================= Phase C: row-parallel LN/FF/collapse =================
        with ExitStack() as phC:
            rowC = phC.enter_context(tc.tile_pool(name="rowC", bufs=4))
            h2T_pool = phC.enter_context(tc.tile_pool(name="h2T", bufs=1))
            psFF = phC.enter_context(tc.tile_pool(name="psFF", bufs=2, space="PSUM"))
            psTrC = phC.enter_context(tc.tile_pool(name="psTrC", bufs=2, space="PSUM"))
            psC1 = phC.enter_context(tc.tile_pool(name="psC1", bufs=2, space="PSUM"))
            psC2 = phC.enter_context(tc.tile_pool(name="psC2", bufs=1, space="PSUM"))
            psFin = phC.enter_context(tc.tile_pool(name="psFin", bufs=1, space="PSUM"))

            from concourse.tile_rust import add_dep_helper as _adh

            def _delay(dma_inst):
                # keep big phase-C weight loads off the DMA queues until the
                # phase-A/B input traffic is done
                _adh(dma_inst.ins, xt_last_dma.ins, sync=True,
                     reason="defer phase-C weight load")
                return dma_inst

            DKC = D // 128  # 8
            wff_sb = wC_pool.tile([128, DKC, D], BF)
            _delay(nc.scalar.dma_start(
                out=wff_sb[:],
                in_=bass.AP(tensor=wff.ap().tensor, offset=0,
                            ap=[[D, 128], [128 * D, DKC], [1, D]])))
            wc1_sb = wC_pool.tile([128, DKC, D], BF)
            _delay(nc.scalar.dma_start(
                out=wc1_sb[:],
                in_=bass.AP(tensor=wc1.ap().tensor, offset=0,
                            ap=[[D, 128], [128 * D, DKC], [1, D]])))
            wc2_sb = wC_pool.tile([128, DKC], BF)
            nc.sync.dma_start(
                out=wc2_sb[:],
                in_=bass.AP(tensor=wc2.ap().tensor, offset=0,
                            ap=[[1, 128], [128, DKC]]))
            bc1_sb = wC_pool.tile([128, DKC], F32)
            nc.sync.dma_start(
                out=bc1_sb[:],
                in_=bass.AP(tensor=bc1.ap().tensor, offset=0,
                            ap=[[1, 128], [128, DKC]]))
            bc2_sb = wC_pool.tile([1, 1], F32)
            nc.sync.dma_start(out=bc2_sb[:], in_=bc2.ap())
            if not trivial_gb:
                gamma_bc = wC_pool.tile([128, D], BF)
                nc.sync.dma_start(out=gamma_bc[:], in_=bcast(gamma, 128, D))
                beta_bc = wC_pool.tile([128, D], BF)
                nc.sync.dma_start(out=beta_bc[:], in_=bcast(beta, 128, D))
            bff_bc = wC_pool.tile([128, D], BF)
            nc.sync.dma_start(out=bff_bc[:], in_=bcast(bff, 128, D))
            # sharded final weights (small: 1MB + 128KB)
            finW = phC.enter_context(tc.tile_pool(name="finW", bufs=1))
            wl1s_sb = finW.tile([128, L // 128, L // NC], BF)
            _delay(nc.scalar.dma_start(
                out=wl1s_sb[:],
                in_=bass.AP(tensor=wl1s.ap().tensor, offset=0,
                            ap=[[L // NC, 128], [128 * (L // NC), L // 128],
                                [1, L // NC]])))
            wl2s_sb = finW.tile([128, 2, OUT], BF)
            _delay(nc.scalar.dma_start(
                out=wl2s_sb[:],
                in_=bass.AP(tensor=wl2s.ap().tensor, offset=0,
                            ap=[[OUT, 128], [128 * OUT, 2], [1, OUT]])))

            # two half-tiles so the c1 matmuls can start once the first four
            # row-tiles are done
            h2T_half = [h2T_pool.tile([128, DKC, RPC // 2], BF, name=f"h2Th{i}")
                        for i in range(2)]
            c2_sb = h2T_pool.tile([1, RPC], BF)

            def layernorm_rows(src, dst, apply_gb):
                """src [128, D] bf16 -> dst bf16 (normalized; opt. *gamma+beta)."""
                stats = rowC.tile([128, 2, nc.vector.BN_STATS_DIM], F32, tag="stats")
                for sg in range(2):
                    nc.vector.bn_stats(stats[:, sg, :], src[:, sg * 512:(sg + 1) * 512])
                mv = rowC.tile([128, nc.vector.BN_AGGR_DIM], F32, tag="mv")
                nc.vector.bn_aggr(mv[:], stats[:])
                sq = rowC.tile([128, 1], F32, tag="sq")
                nc.scalar.activation(sq[:], mv[:, 1:2], AF.Sqrt, bias=eps_sb[:], scale=1.0)
                rstd = rowC.tile([128, 1], F32, tag="rstd")
                nc.vector.reciprocal(rstd[:], sq[:])
                if apply_gb and not trivial_gb:
                    z = rowC.tile([128, D], BF, tag="zf")
                    nc.vector.tensor_scalar(z[:], src[:], mv[:, 0:1], rstd[:],
                                            op0=OP.subtract, op1=OP.mult)
                    zg = rowC.tile([128, D], BF, tag="zg")
                    nc.vector.tensor_mul(zg[:], z[:], gamma_bc[:])
                    nc.vector.tensor_add(dst[:], zg[:], beta_bc[:])
                else:
                    nc.vector.tensor_scalar(dst[:], src[:], mv[:, 0:1], rstd[:],
                                            op0=OP.subtract, op1=OP.mult)

            for t in range(RPC // 128):
                b, e = t // 2, t % 2
                # rows of this tile: batch b, L-positions [i*LPC + e*128 + p)
                # (core-specific x rows arrive pre-sliced via `xrows`)
                ctx_t = rowC.tile([128, H, DH], BF, tag="ctx_t")
                nc.sync.dma_start(
                    out=ctx_t[:],
                    in_=bass.AP(tensor=a2a_out.ap().tensor,
                                offset=(b * L + e * 128) * DH,
                                ap=[[DH, 128], [LPC * DH, H], [1, DH]]))
                x_t = rowC.tile([128, D], BF, tag="x_t")
                nc.sync.dma_start(out=x_t[:], in_=xrows.ap()[t * 128:(t + 1) * 128, :])
                s_t = rowC.tile([128, D], BF, tag="s_t")
                nc.gpsimd.tensor_add(s_t[:], x_t[:], ctx_t[:].rearrange("p h d -> p (h d)"))
                h1b = rowC.tile([128, D], BF, tag="h1b")
                layernorm_rows(s_t, h1b, apply_gb=True)
                # h1^T for the ff matmul (8 transposes -> one psum bank -> one copy)
                h1T = rowC.tile([128, DKC, 128], BF, tag="h1T")
                tps1 = psTrC.tile([128, DKC, 128], BF, tag="htr", name=f"h1tr{t}")
                for kc in range(DKC):
                    nc.tensor.transpose(tps1[:, kc, :],
                                        h1b[:, kc * 128:(kc + 1) * 128], ident[:])
                nc.vector.tensor_copy(h1T[:], tps1[:])
                # ff natural [128 rows, D]
                f_t = rowC.tile([128, D], BF, tag="f_t")
                for dc in range(2):
                    fps = psFF.tile([128, 512], F32, tag="fps")
                    for kc in range(DKC):
                        nc.tensor.matmul(fps[:], h1T[:, kc, :],
                                         wff_sb[:, kc, dc * 512:(dc + 1) * 512],
                                         start=(kc == 0), stop=(kc == DKC - 1))
                    tmp = rowC.tile([128, 512], BF, tag="fftmp")
                    nc.vector.tensor_add(tmp[:], fps[:], bff_bc[:, dc * 512:(dc + 1) * 512])
                    nc.scalar.activation(f_t[:, dc * 512:(dc + 1) * 512], tmp[:],
                                         AF.Relu, bias=0.0, scale=1.0)
                s2_t = rowC.tile([128, D], BF, tag="s2_t")
                nc.gpsimd.tensor_add(s2_t[:], h1b[:], f_t[:])
                h2b = rowC.tile([128, D], BF, tag="h2b")
                layernorm_rows(s2_t, h2b, apply_gb=False)  # gamma/beta folded into wc1
                tps2 = psTrC.tile([128, DKC, 128], BF, tag="htr", name=f"h2tr{t}")
                for kc in range(DKC):
                    nc.tensor.transpose(tps2[:, kc, :],
                                        h2b[:, kc * 128:(kc + 1) * 128], ident[:])
                nc.vector.tensor_copy(
                    h2T_half[t // 4][:, :, (t % 4) * 128:(t % 4 + 1) * 128],
                    tps2[:])

            # c1^T = relu(wc1'^T h2 + bc1') ; then c2 = relu(c1 @ wc2 + bc2)
            c1T = h2T_pool.tile([128, DKC, RPC], BF)
            for rc in range(RPC // 512):
                for fc in range(DKC):
                    cps = psC1.tile([128, 512], F32, tag="c1ps")
                    for kc in range(DKC):
                        nc.tensor.matmul(cps[:], wc1_sb[:, kc, fc * 128:(fc + 1) * 128],
                                         h2T_half[rc][:, kc, :],
                                         start=(kc == 0), stop=(kc == DKC - 1))
                    nc.scalar.activation(c1T[:, fc, rc * 512:(rc + 1) * 512], cps[:],
                                         AF.Relu, bias=bc1_sb[:, fc:fc + 1], scale=1.0)
            for rc in range(RPC // 512):
                c2ps = psC2.tile([1, 512], F32, tag="c2ps")
                for kc in range(DKC):
                    nc.tensor.matmul(c2ps[:], wc2_sb[:, kc:kc + 1],
                                     c1T[:, kc, rc * 512:(rc + 1) * 512],
                                     start=(kc == 0), stop=(kc == DKC - 1))
                nc.scalar.activation(c2_sb[0:1, rc * 512:(rc + 1) * 512], c2ps[:],
                                     AF.Relu, bias=bc2_sb[0:1, :], scale=1.0)
            nc.sync.dma_start(out=c_ag_in.ap().rearrange("(o n) -> o n", o=1),
                              in_=c2_sb[0:1, :])
            nc.gpsimd.collective_compute(
                "AllGather", OP.bypass,
                ins=[c_ag_in.ap()], outs=[c_ag_out.ap()], replica_groups=RG)

            # final (column-sharded): partial = relu(c @ wl1s + bl1s) @ wl2s,
            # AllReduce partials, + bl2 (identical on every core)
            cTb = rowC.tile([128, 2, NC, B], BF, tag="cTb")
            for e in range(2):
                nc.sync.dma_start(
                    out=cTb[:, e, :, :],
                    in_=bass.AP(tensor=c_ag_out.ap().tensor, offset=e * 128,
                                ap=[[1, 128], [RPC, NC], [LPC, B]]))
            bl1_bc = finW.tile([B, L // NC], BF)
            nc.sync.dma_start(out=bl1_bc[:], in_=bcast(bl1s, B, L // NC))
            bl2_bc = finW.tile([B, OUT], F32)
            nc.sync.dma_start(out=bl2_bc[:], in_=bcast(bl2, B, OUT))
            fps = psFin.tile([B, L // NC], F32, tag="finps")
            for kc in range(L // 128):
                nc.tensor.matmul(fps[:], cTb[:, kc % 2, kc // 2, :],
                                 wl1s_sb[:, kc, :],
                                 start=(kc == 0), stop=(kc == L // 128 - 1))
            tmp = rowC.tile([B, L // NC], F32, tag="fintmp")
            nc.vector.tensor_add(tmp[:], fps[:], bl1_bc[:])
            c1fs = rowC.tile([B, L // NC], BF, tag="c1fs")
            nc.vector.tensor_scalar_max(c1fs[:], tmp[:], 0.0)
            c1fT = rowC.tile([128, 2, B], BF, tag="c1fT")
            for j in range(2):
                tps = psTrC.tile([128, B], BF, tag="htr")
                nc.tensor.transpose(tps[:], c1fs[0:B, j * 128:(j + 1) * 128],
                                    ident[0:B, 0:B])
                nc.vector.tensor_copy(c1fT[:, j, :], tps[:])
            ops = psFin.tile([B, OUT], F32, tag="finps")
            for kc in range(2):
                nc.tensor.matmul(ops[:], c1fT[:, kc, :], wl2s_sb[:, kc, :],
                                 start=(kc == 0), stop=(kc == 1))
            part_sb = rowC.tile([B, OUT], F32, tag="part_sb")
            nc.vector.tensor_copy(part_sb[:], ops[:])
            nc.sync.dma_start(out=ar_in.ap(), in_=part_sb[:])
            nc.gpsimd.collective_compute(
                "AllReduce", OP.add,
                ins=[ar_in.ap()], outs=[ar_out.ap()], replica_groups=RG)
            red_out = rowC.tile([B, OUT], F32, tag="red_out")
            nc.sync.dma_start(out=red_out[:], in_=ar_out.ap())
            out_f = rowC.tile([B, OUT], F32, tag="out_f")
            nc.vector.tensor_add(out_f[:], red_out[:], bl2_bc[:])
            nc.sync.dma_start(out=out.ap(), in_=out_f[:])

    nc.compile()
    return nc


def _to_bf16(a):
    return np.asarray(a, dtype=np.float32).astype(ml_dtypes.bfloat16)


def kernel(**inputs):
    from concourse.bass_utils import run_bass_kernel_spmd

    gamma_np0 = np.asarray(inputs["gamma"], dtype=np.float32)
    beta_np0 = np.asarray(inputs["beta"], dtype=np.float32)
    trivial_gb = bool(np.all(gamma_np0 == 1.0) and np.all(beta_np0 == 0.0))
    key = ("nc", trivial_gb)
    if key not in _CACHE:
        _CACHE[key] = _build_nc(trivial_gb=trivial_gb)
    nc = _CACHE[key]

    x = np.asarray(inputs["x"], dtype=np.float32).reshape(N, D)
    isq = 1.0 / math.sqrt(DH)
    gamma_np = np.asarray(inputs["gamma"], dtype=np.float32)
    beta_np = np.asarray(inputs["beta"], dtype=np.float32)
    wc1_np = np.asarray(inputs["wc1"], dtype=np.float32)
    bc1_np = np.asarray(inputs["bc1"], dtype=np.float32)
    # fold LN2's gamma/beta into the c1 projection (h2 feeds only this matmul)
    wc1_f = gamma_np[:, None] * wc1_np
    bc1_f = bc1_np + beta_np @ wc1_np

    xT_bf = np.ascontiguousarray(_to_bf16(x).T)
    shared = dict(
        xT=xT_bf,
        wff=_to_bf16(inputs["wff"]),
        bff=_to_bf16(inputs["bff"]),
        gamma=_to_bf16(gamma_np), beta=_to_bf16(beta_np),
        wc1=_to_bf16(wc1_f), bc1=bc1_f.astype(np.float32),
        wc2=_to_bf16(np.asarray(inputs["wc2"]).reshape(D)),
        bc2=np.asarray(inputs["bc2"], np.float32).reshape(1),
        bl2=np.asarray(inputs["bl2"], np.float32),
    )
    wl1_np = np.asarray(inputs["wl1"], np.float32)
    bl1_np = np.asarray(inputs["bl1"], np.float32)
    wl2_np = np.asarray(inputs["wl2"], np.float32)
    wq = np.asarray(inputs["wq"], np.float32) * isq
    bq = np.asarray(inputs["bq"], np.float32) * isq
    wk = np.asarray(inputs["wk"], np.float32)
    bk = np.asarray(inputs["bk"], np.float32)
    wv = np.asarray(inputs["wv"], np.float32)
    bv = np.asarray(inputs["bv"], np.float32)

    in_maps = []
    for i in range(NC):
        sl = slice(i * DH, (i + 1) * DH)
        wqkv_i = np.stack([wq[:, sl], wk[:, sl], wv[:, sl]])
        bqkv_i = np.stack([bq[sl], bk[sl], bv[sl]])
        # rows this core owns after the A2A: for each batch b, L-positions
        # [i*LPC, (i+1)*LPC) -> 8 row-tiles of 128 = (b, e) pairs
        xr = np.concatenate([
            x[b * L + i * LPC: b * L + (i + 1) * LPC, :] for b in range(B)
        ])  # [RPC, D] ordered (b, l-within-block)
        csl = slice(i * (L // NC), (i + 1) * (L // NC))
        in_maps.append(dict(
            shared,
            wqkv=_to_bf16(wqkv_i),
            bqkv=bqkv_i.astype(np.float32),
            xrows=_to_bf16(xr),
            wl1s=_to_bf16(wl1_np[:, csl]),
            bl1s=_to_bf16(bl1_np[csl]),
            wl2s=_to_bf16(wl2_np[csl, :]),
        ))

    res = run_bass_kernel_spmd(nc, in_maps, core_ids=list(range(NC)))
    return np.asarray(res.results[0]["out"], dtype=np.float32)



# revision 14
# speedup vs baseline: 1.0765x; 1.0765x over previous
"""Distributed Trainium2 kernel for nn_Attention_64854006169830.

Strategy (8 NeuronCores, SPMD):
  - QKV + attention: head-parallel (core i computes head i for all B*L rows),
    activations kept feature-major so every matmul uses natural weight
    layouts. QKV projections run in fp8-e4m3 DoubleRow mode (2x PE
    throughput); weights are pre-scaled by 32 on host so they sit in e4m3's
    normal range, and the scale is folded back out via the exp() scale and
    the denominator ones-vector.
  - Softmax on transposed scores (keys on partitions): exp on ACT writes the
    unnormalized probabilities straight to fp8 (shifted by a constant -C;
    shift cancels in normalization). Denominators and attn@V both run as
    fp8 DoubleRow matmuls on PE; normalization happens after the
    PE-transpose back to row-major.
  - ctx redistribution head-shard -> row-shard via per-batch AllToAll
    (overlapped with attention of later batches).
  - LN + FF + collapse(d->1): row-parallel, bf16 (fp8 fails the error gate
    here). Residual adds on DVE (gpsimd is ~6x slower and serialized the
    pipeline).
  - Tail: each core computes its rows' contribution to c @ wl1 directly
    (row-sharded wl1), one 32KB AllReduce sums the partials, then every
    core redundantly computes the tiny l2 matmul. This replaces the old
    AllGather + AllReduce pair and most of the tail latency.
Compute dtype: bf16/fp8-e4m3 (f32 accumulation in PSUM); ~0.6% rel err vs
the float32 reference (gate is 2e-2).
"""
import sys
import math

for _p in ("/opt/trn_rl_repo", "/opt/trn_rl_repo/concourse"):
    if _p not in sys.path:
        sys.path.insert(0, _p)

import numpy as np
import ml_dtypes

B, L, D, H, OUT = 4, 2048, 1024, 8, 256
DH = D // H          # 128
N = B * L            # 8192 rows
NC = 8               # cores
RPC = N // NC        # 1024 rows per core (as 4 batches x 256 L-positions)
LPC = L // NC        # 256 L-positions per core per batch
EPS = 1e-12
SCL = 32.0           # fp8 weight pre-scale (host); folded out on device
C_SHIFT = 2.0        # softmax exp shift; cancels in normalization

_CACHE = {}


def _build_nc(trivial_gb=False):
    import concourse.bass as bass
    import concourse.tile as tile
    from concourse import bacc, mybir
    from concourse.masks import make_identity

    BF = mybir.dt.bfloat16
    F8 = mybir.dt.float8e4
    F32 = mybir.dt.float32
    AF = mybir.ActivationFunctionType
    OP = mybir.AluOpType
    DR = mybir.MatmulPerfMode.DoubleRow

    nc = bacc.Bacc("TRN2", debug=False, num_devices=NC)

    # ---- parameters (per-core values supplied via in_maps) ----
    xT = nc.dram_tensor("xT", [D, N], F8, kind="ExternalInput")
    xrows = nc.dram_tensor("xrows", [RPC, D], BF, kind="ExternalInput")
    wqkv = nc.dram_tensor("wqkv", [3, D, DH], F8, kind="ExternalInput")
    bqkv = nc.dram_tensor("bqkv", [3, DH], F32, kind="ExternalInput")
    wff = nc.dram_tensor("wff", [D, D], BF, kind="ExternalInput")
    bff = nc.dram_tensor("bff", [D], BF, kind="ExternalInput")
    gamma = nc.dram_tensor("gamma", [D], BF, kind="ExternalInput")
    beta = nc.dram_tensor("beta", [D], BF, kind="ExternalInput")
    wc1 = nc.dram_tensor("wc1", [D, D], BF, kind="ExternalInput")   # gamma-folded
    bc1 = nc.dram_tensor("bc1", [D], F32, kind="ExternalInput")     # beta-folded
    wc2 = nc.dram_tensor("wc2", [D], BF, kind="ExternalInput")
    bc2 = nc.dram_tensor("bc2", [1], F32, kind="ExternalInput")
    wl1r = nc.dram_tensor("wl1r", [LPC, L], BF, kind="ExternalInput")  # row-shard
    bl1t4 = nc.dram_tensor("bl1t4", [L, B], F32, kind="ExternalInput")
    wl2f = nc.dram_tensor("wl2f", [L, OUT], BF, kind="ExternalInput")  # full
    bl2 = nc.dram_tensor("bl2", [OUT], F32, kind="ExternalInput")
    out = nc.dram_tensor("out", [B, OUT], F32, kind="ExternalOutput")

    # ---- internal DRAM ----
    a2a_in = nc.dram_tensor("a2a_in", [N, DH], BF)
    a2a_out = nc.dram_tensor("a2a_out", [N, DH], BF)
    sums_hbm = nc.dram_tensor("sums_hbm", [N], F32)
    c_hbm = nc.dram_tensor("c_hbm", [RPC], BF)
    l1p_in = nc.dram_tensor("l1p_in", [L, B], F32)
    l1p_out = nc.dram_tensor("l1p_out", [L, B], F32, addr_space="Shared")

    def bcast(dram_handle, parts, free):
        """Broadcast a [free] DRAM vector across `parts` partitions."""
        ap = dram_handle.ap()
        return bass.AP(tensor=ap.tensor, offset=0, ap=[[0, parts], [1, free]])

    RG = [list(range(NC))]

    from contextlib import ExitStack

    with tile.TileContext(nc) as tc, ExitStack() as root:
        glob = root.enter_context(tc.tile_pool(name="glob", bufs=1))
        ident = glob.tile([128, 128], BF)
        make_identity(nc, ident[:])
        ones2 = glob.tile([128, 2, 128], F8)
        nc.vector.memset(ones2[:], SCL)  # folds the v-scale back out of ctx
        eps_sb = glob.tile([128, 1], F32)
        nc.vector.memset(eps_sb[:], EPS)
        negc_sb = glob.tile([128, 1], F32)
        nc.vector.memset(negc_sb[:], -C_SHIFT)

        # Phase-C weight pool carved out first so its loads never overlap
        # (in address space) with the big transient phase-A/B tiles.
        wC_pool = root.enter_context(tc.tile_pool(name="wC", bufs=1))

        phAB = root.enter_context(ExitStack())
        qkv_pool = phAB.enter_context(tc.tile_pool(name="qkv", bufs=1))
        # persistent through phases A+B; per-batch tiles so attention on
        # batch b can start as soon as batch b's QKV is done
        qkvT = [qkv_pool.tile([128, 2, L], BF, name=f"qkvT{b}") for b in range(B)]
        vnat = [qkv_pool.tile([128, L // 128, DH], F8, name=f"vnat{b}")
                for b in range(B)]

        # ================= Phase A: QKV^T (fp8 DoubleRow) =================
        with ExitStack() as phA:
            xt_pool = phA.enter_context(tc.tile_pool(name="xt", bufs=2))
            wq_pool = phA.enter_context(tc.tile_pool(name="wqkv", bufs=1))
            psA = phA.enter_context(tc.tile_pool(name="psA", bufs=6, space="PSUM"))
            psTrA = phA.enter_context(tc.tile_pool(name="psTrA", bufs=2, space="PSUM"))
            vstage_pool = phA.enter_context(tc.tile_pool(name="vstage", bufs=2))

            # weights first (tiny) so the first matmuls aren't stuck behind
            # the 8MB x^T load in the DMA queues
            wq_sb = wq_pool.tile([128, 3, D // 128, DH], F8)
            nc.sync.dma_start(
                out=wq_sb[:],
                in_=bass.AP(tensor=wqkv.ap().tensor, offset=0,
                            ap=[[DH, 128], [D * DH, 3], [128 * DH, D // 128], [1, DH]]))
            bq_sb = wq_pool.tile([128, 3], F32)
            nc.sync.dma_start(
                out=bq_sb[:],
                in_=bass.AP(tensor=bqkv.ap().tensor, offset=0,
                            ap=[[1, 128], [DH, 3]]))

            # row-group-major so compute on group g starts right after its DMA
            for rg in range(4):
                xt = xt_pool.tile([128, D // 128, L], F8, tag="xt",
                                  name=f"xt{rg}")
                for kc in range(D // 128):
                    xt_last_dma = nc.sync.dma_start(
                        out=xt[:, kc, :],
                        in_=xT.ap()[kc * 128:(kc + 1) * 128,
                                    rg * 2048:(rg + 1) * 2048])
                for s in range(3):
                    pst = [psA.tile([128, 512], F32, tag="qkvps", name=f"qkvps{rg}_{s}_{j}")
                           for j in range(4)]
                    for k2 in range(D // 256):
                        for r4 in range(4):
                            nc.tensor.matmul(
                                pst[r4][:], wq_sb[:, s, 2 * k2:2 * k2 + 2, :],
                                xt[:, 2 * k2:2 * k2 + 2, r4 * 512:(r4 + 1) * 512],
                                start=(k2 == 0), stop=(k2 == D // 256 - 1),
                                perf_mode=DR)
                    for r4 in range(4):
                        if s < 2:
                            nc.vector.tensor_scalar_add(
                                qkvT[rg][:, s, r4 * 512:(r4 + 1) * 512], pst[r4][:],
                                bq_sb[:, s:s + 1])
                        else:
                            # v: bias-add to staging, then PE-transpose into
                            # row-major vnat (fp8 via the psum->sbuf copy)
                            vstage = vstage_pool.tile([128, 512], BF, tag="vstage",
                                                      name=f"vst{rg}_{r4}")
                            nc.vector.tensor_scalar_add(
                                vstage[:], pst[r4][:], bq_sb[:, s:s + 1])
                            tps = psTrA.tile([128, 4, 128], BF, tag="vtr",
                                             name=f"vtr{rg}_{r4}")
                            for j in range(4):
                                nc.tensor.transpose(
                                    tps[:, j, :], vstage[:, j * 128:(j + 1) * 128],
                                    ident[:])
                            nc.vector.tensor_copy(
                                vnat[rg][:, r4 * 4:(r4 + 1) * 4, :], tps[:])

        # ================= Phase B: attention per batch =================
        with ExitStack() as phB:
            pT_pool = phB.enter_context(tc.tile_pool(name="pT", bufs=2))
            ctxT_pool = phB.enter_context(tc.tile_pool(name="ctxT", bufs=2))
            sums_pool = phB.enter_context(tc.tile_pool(name="sums", bufs=1))
            recip_pool = phB.enter_context(tc.tile_pool(name="recip", bufs=2))
            norm_pool = phB.enter_context(tc.tile_pool(name="norm", bufs=3))
            psS = phB.enter_context(tc.tile_pool(name="psS", bufs=2, space="PSUM"))
            psC = phB.enter_context(tc.tile_pool(name="psC", bufs=2, space="PSUM"))
            psD = phB.enter_context(tc.tile_pool(name="psD", bufs=1, space="PSUM"))
            psTrB = phB.enter_context(tc.tile_pool(name="psTrB", bufs=1, space="PSUM"))

            KCB = L // 128  # 16 key chunks per batch
            ESC = 1.0 / (SCL * SCL)  # undo the q/k fp8 pre-scale inside exp

            for b in range(B):
                ctxT_sb = ctxT_pool.tile([128, L], BF, tag="ctxT")
                for qc in range(L // 1024):
                    pT = pT_pool.tile([128, KCB, 1024], F8, tag="pT")
                    sums_sb = sums_pool.tile([1, 1024], F32, tag="sums", bufs=2,
                                             name=f"sums{b}_{qc}")
                    q0 = qc * 1024
                    for kc in range(KCB):
                        sps = psS.tile([128, 1024], F32, tag="sps")
                        for hh in range(2):
                            nc.tensor.matmul(
                                sps[:, hh * 512:(hh + 1) * 512],
                                qkvT[b][:, 1, kc * 128:(kc + 1) * 128],
                                qkvT[b][:, 0, q0 + hh * 512: q0 + (hh + 1) * 512],
                                start=True, stop=True)
                        # unnormalized probs straight to fp8; shift by -C
                        # (cancels in normalization), scale undoes SCL^2
                        nc.scalar.activation(pT[:, kc, :], sps[:], AF.Exp,
                                             bias=negc_sb[:], scale=ESC)
                    cps2 = [psC.tile([128, 512], F32, tag="cps", name=f"cps{b}_{qc}_{h}")
                            for h in range(2)]
                    for k2 in range(KCB // 2):
                        for hh in range(2):
                            nc.tensor.matmul(cps2[hh][:],
                                             vnat[b][:, 2 * k2:2 * k2 + 2, :],
                                             pT[:, 2 * k2:2 * k2 + 2,
                                                hh * 512:(hh + 1) * 512],
                                             start=(k2 == 0), stop=(k2 == KCB // 2 - 1),
                                             perf_mode=DR)
                    for hh in range(2):
                        hsl = slice(hh * 512, (hh + 1) * 512)
                        cps = cps2[hh]
                        # softmax denominators: fp8 DoubleRow ones-matmuls
                        # (full 128-wide weights: DR rejects sub-128 PE tiles;
                        # all 128 output partitions hold the same sum)
                        sps2 = psD.tile([128, 512], F32, tag="sps2")
                        for k2 in range(KCB // 2):
                            nc.tensor.matmul(sps2[:], ones2[:],
                                             pT[:, 2 * k2:2 * k2 + 2, hsl],
                                             start=(k2 == 0), stop=(k2 == KCB // 2 - 1),
                                             perf_mode=DR)
                        nc.vector.tensor_copy(
                            ctxT_sb[:, qc * 1024 + hh * 512: qc * 1024 + (hh + 1) * 512],
                            cps[:])
                        nc.vector.tensor_copy(sums_sb[:, hh * 512:(hh + 1) * 512],
                                              sps2[0:1, :])
                    # per-qc epilogue: recip roundtrip, transpose back to
                    # row-major, normalize, store this 1024-row slice
                    q_hbm = b * L + qc * 1024
                    nc.sync.dma_start(
                        out=sums_hbm.ap()[q_hbm:q_hbm + 1024].rearrange(
                            "(o n) -> o n", o=1),
                        in_=sums_sb[0:1, :])
                    rraw = recip_pool.tile([128, 8], F32, tag="rraw",
                                           name=f"rraw{b}_{qc}")
                    nc.sync.dma_start(
                        out=rraw[:],
                        in_=sums_hbm.ap()[q_hbm:q_hbm + 1024].rearrange(
                            "(j p) -> p j", p=128))
                    rcols = recip_pool.tile([128, 8], F32, tag="rcols",
                                            name=f"rcols{b}_{qc}")
                    nc.vector.reciprocal(rcols[:], rraw[:])
                    nrm = norm_pool.tile([128, 8, DH], BF, tag="nrm",
                                         name=f"nrm{b}_{qc}")
                    tpsq = psTrB.tile([128, 8, 128], BF, tag="ctr",
                                      name=f"ctr{b}_{qc}")
                    for j in range(8):
                        nc.tensor.transpose(
                            tpsq[:, j, :],
                            ctxT_sb[:, qc * 1024 + j * 128: qc * 1024 + (j + 1) * 128],
                            ident[:])
                    for j in range(8):
                        nc.vector.tensor_scalar_mul(nrm[:, j, :], tpsq[:, j, :],
                                                    rcols[:, j:j + 1])
                    nc.sync.dma_start(
                        out=bass.AP(tensor=a2a_in.ap().tensor, offset=q_hbm * DH,
                                    ap=[[DH, 128], [128 * DH, 8], [1, DH]]),
                        in_=nrm[:])
                nc.gpsimd.collective_compute(
                    "AllToAll", OP.bypass,
                    ins=[a2a_in.ap()[b * L:(b + 1) * L, :]],
                    outs=[a2a_out.ap()[b * L:(b + 1) * L, :]],
                    replica_groups=RG)

        phAB.close()  # release qkvT/vnat space for phase C

        # ================= Phase C: row-parallel LN/FF/collapse =================
        with ExitStack() as phC:
            rowC = phC.enter_context(tc.tile_pool(name="rowC", bufs=4))
            h2T_pool = phC.enter_context(tc.tile_pool(name="h2T", bufs=1))
            psFF = phC.enter_context(tc.tile_pool(name="psFF", bufs=2, space="PSUM"))
            psTrC = phC.enter_context(tc.tile_pool(name="psTrC", bufs=2, space="PSUM"))
            psC1 = phC.enter_context(tc.tile_pool(name="psC1", bufs=2, space="PSUM"))
            # c2 / l1-partial / final psum tiles are sequential; share one bank
            psSm = phC.enter_context(tc.tile_pool(name="psSm", bufs=1, space="PSUM"))

            from concourse.tile_rust import add_dep_helper as _adh

            def _delay(dma_inst):
                # keep big phase-C weight loads off the DMA queues until the
                # phase-A/B input traffic is done
                _adh(dma_inst.ins, xt_last_dma.ins, sync=True,
                     reason="defer phase-C weight load")
                return dma_inst

            DKC = D // 128  # 8
            wff_sb = wC_pool.tile([128, DKC, D], BF)
            _delay(nc.scalar.dma_start(
                out=wff_sb[:],
                in_=bass.AP(tensor=wff.ap().tensor, offset=0,
                            ap=[[D, 128], [128 * D, DKC], [1, D]])))
            wc1_sb = wC_pool.tile([128, DKC, D], BF)
            _delay(nc.scalar.dma_start(
                out=wc1_sb[:],
                in_=bass.AP(tensor=wc1.ap().tensor, offset=0,
                            ap=[[D, 128], [128 * D, DKC], [1, D]])))
            wc2_sb = wC_pool.tile([128, DKC], BF)
            nc.sync.dma_start(
                out=wc2_sb[:],
                in_=bass.AP(tensor=wc2.ap().tensor, offset=0,
                            ap=[[1, 128], [128, DKC]]))
            bc1_sb = wC_pool.tile([128, DKC], F32)
            nc.sync.dma_start(
                out=bc1_sb[:],
                in_=bass.AP(tensor=bc1.ap().tensor, offset=0,
                            ap=[[1, 128], [128, DKC]]))
            bc2_sb = wC_pool.tile([1, 1], F32)
            nc.sync.dma_start(out=bc2_sb[:], in_=bc2.ap())
            if not trivial_gb:
                gamma_bc = wC_pool.tile([128, D], BF)
                nc.sync.dma_start(out=gamma_bc[:], in_=bcast(gamma, 128, D))
                beta_bc = wC_pool.tile([128, D], BF)
                nc.sync.dma_start(out=beta_bc[:], in_=bcast(beta, 128, D))
            bff_bc = wC_pool.tile([128, D], BF)
            nc.sync.dma_start(out=bff_bc[:], in_=bcast(bff, 128, D))
            # final-stage weights: wl1 row-shard (1MB) + full wl2 (1MB)
            finW = phC.enter_context(tc.tile_pool(name="finW", bufs=1))
            wl1r_sb = finW.tile([128, 2, L], BF)
            _delay(nc.scalar.dma_start(
                out=wl1r_sb[:],
                in_=bass.AP(tensor=wl1r.ap().tensor, offset=0,
                            ap=[[L, 128], [128 * L, 2], [1, L]])))
            wl2_sb = finW.tile([128, L // 128, OUT], BF)
            _delay(nc.scalar.dma_start(
                out=wl2_sb[:],
                in_=bass.AP(tensor=wl2f.ap().tensor, offset=0,
                            ap=[[OUT, 128], [128 * OUT, L // 128], [1, OUT]])))
            bl1t_sb = finW.tile([128, L // 128, B], F32)
            nc.sync.dma_start(
                out=bl1t_sb[:],
                in_=bass.AP(tensor=bl1t4.ap().tensor, offset=0,
                            ap=[[B, 128], [128 * B, L // 128], [1, B]]))
            bl2_bc = finW.tile([B, OUT], F32)
            nc.sync.dma_start(out=bl2_bc[:], in_=bcast(bl2, B, OUT))

            # two half-tiles so the c1 matmuls can start once the first four
            # row-tiles are done
            h2T_half = [h2T_pool.tile([128, DKC, RPC // 2], BF, name=f"h2Th{i}")
                        for i in range(2)]
            c2_sb = h2T_pool.tile([1, RPC], BF)

            def layernorm_rows(src, dst, apply_gb):
                """src [128, D] bf16 -> dst bf16 (normalized; opt. *gamma+beta)."""
                stats = rowC.tile([128, 2, nc.vector.BN_STATS_DIM], F32, tag="stats")
                for sg in range(2):
                    nc.vector.bn_stats(stats[:, sg, :], src[:, sg * 512:(sg + 1) * 512])
                mv = rowC.tile([128, nc.vector.BN_AGGR_DIM], F32, tag="mv")
                nc.vector.bn_aggr(mv[:], stats[:])
                sq = rowC.tile([128, 1], F32, tag="sq")
                nc.scalar.activation(sq[:], mv[:, 1:2], AF.Sqrt, bias=eps_sb[:], scale=1.0)
                rstd = rowC.tile([128, 1], F32, tag="rstd")
                nc.vector.reciprocal(rstd[:], sq[:])
                if apply_gb and not trivial_gb:
                    z = rowC.tile([128, D], BF, tag="zf")
                    nc.vector.tensor_scalar(z[:], src[:], mv[:, 0:1], rstd[:],
                                            op0=OP.subtract, op1=OP.mult)
                    zg = rowC.tile([128, D], BF, tag="zg")
                    nc.vector.tensor_mul(zg[:], z[:], gamma_bc[:])
                    nc.vector.tensor_add(dst[:], zg[:], beta_bc[:])
                else:
                    nc.vector.tensor_scalar(dst[:], src[:], mv[:, 0:1], rstd[:],
                                            op0=OP.subtract, op1=OP.mult)

            for t in range(RPC // 128):
                b, e = t // 2, t % 2
                # rows of this tile: batch b, L-positions [i*LPC + e*128 + p)
                # (core-specific x rows arrive pre-sliced via `xrows`)
                ctx_t = rowC.tile([128, H, DH], BF, tag="ctx_t")
                nc.sync.dma_start(
                    out=ctx_t[:],
                    in_=bass.AP(tensor=a2a_out.ap().tensor,
                                offset=(b * L + e * 128) * DH,
                                ap=[[DH, 128], [LPC * DH, H], [1, DH]]))
                x_t = rowC.tile([128, D], BF, tag="x_t")
                nc.sync.dma_start(out=x_t[:], in_=xrows.ap()[t * 128:(t + 1) * 128, :])
                s_t = rowC.tile([128, D], BF, tag="s_t")
                nc.vector.tensor_add(s_t[:], x_t[:], ctx_t[:].rearrange("p h d -> p (h d)"))
                h1b = rowC.tile([128, D], BF, tag="h1b")
                layernorm_rows(s_t, h1b, apply_gb=True)
                # h1^T for the ff matmul (8 transposes -> one psum bank -> one copy)
                h1T = rowC.tile([128, DKC, 128], BF, tag="h1T")
                tps1 = psTrC.tile([128, DKC, 128], BF, tag="htr", name=f"h1tr{t}")
                for kc in range(DKC):
                    nc.tensor.transpose(tps1[:, kc, :],
                                        h1b[:, kc * 128:(kc + 1) * 128], ident[:])
                nc.vector.tensor_copy(h1T[:], tps1[:])
                # ff natural [128 rows, D]
                f_t = rowC.tile([128, D], BF, tag="f_t")
                for dc in range(2):
                    fps = psFF.tile([128, 512], F32, tag="fps")
                    for kc in range(DKC):
                        nc.tensor.matmul(fps[:], h1T[:, kc, :],
                                         wff_sb[:, kc, dc * 512:(dc + 1) * 512],
                                         start=(kc == 0), stop=(kc == DKC - 1))
                    tmp = rowC.tile([128, 512], BF, tag="fftmp")
                    nc.vector.tensor_add(tmp[:], fps[:], bff_bc[:, dc * 512:(dc + 1) * 512])
                    nc.scalar.activation(f_t[:, dc * 512:(dc + 1) * 512], tmp[:],
                                         AF.Relu, bias=0.0, scale=1.0)
                s2_t = rowC.tile([128, D], BF, tag="s2_t")
                nc.vector.tensor_add(s2_t[:], h1b[:], f_t[:])
                h2b = rowC.tile([128, D], BF, tag="h2b")
                layernorm_rows(s2_t, h2b, apply_gb=False)  # gamma/beta folded into wc1
                tps2 = psTrC.tile([128, DKC, 128], BF, tag="htr", name=f"h2tr{t}")
                for kc in range(DKC):
                    nc.tensor.transpose(tps2[:, kc, :],
                                        h2b[:, kc * 128:(kc + 1) * 128], ident[:])
                nc.vector.tensor_copy(
                    h2T_half[t // 4][:, :, (t % 4) * 128:(t % 4 + 1) * 128],
                    tps2[:])

            # c1^T = relu(wc1'^T h2 + bc1') ; then c2 = relu(c1 @ wc2 + bc2)
            c1T = h2T_pool.tile([128, DKC, RPC], BF)
            for rc in range(RPC // 512):
                for fc in range(DKC):
                    cps = psC1.tile([128, 512], F32, tag="c1ps")
                    for kc in range(DKC):
                        nc.tensor.matmul(cps[:], wc1_sb[:, kc, fc * 128:(fc + 1) * 128],
                                         h2T_half[rc][:, kc, :],
                                         start=(kc == 0), stop=(kc == DKC - 1))
                    nc.scalar.activation(c1T[:, fc, rc * 512:(rc + 1) * 512], cps[:],
                                         AF.Relu, bias=bc1_sb[:, fc:fc + 1], scale=1.0)
            for rc in range(RPC // 512):
                c2ps = psSm.tile([1, 512], F32, tag="sm", name=f"c2ps{rc}")
                for kc in range(DKC):
                    nc.tensor.matmul(c2ps[:], wc2_sb[:, kc:kc + 1],
                                     c1T[:, kc, rc * 512:(rc + 1) * 512],
                                     start=(kc == 0), stop=(kc == DKC - 1))
                nc.scalar.activation(c2_sb[0:1, rc * 512:(rc + 1) * 512], c2ps[:],
                                     AF.Relu, bias=bc2_sb[0:1, :], scale=1.0)

            # ---- tail: partial c @ wl1 on local rows, one AllReduce, then
            # every core redundantly computes the tiny l2 matmul ----
            nc.sync.dma_start(out=c_hbm.ap().rearrange("(o n) -> o n", o=1),
                              in_=c2_sb[0:1, :])
            cT_sb = rowC.tile([128, B, 2], BF, tag="cT_sb")
            nc.sync.dma_start(
                out=cT_sb[:],
                in_=bass.AP(tensor=c_hbm.ap().tensor, offset=0,
                            ap=[[1, 128], [256, B], [128, 2]]))
            l1ps = psSm.tile([128, L // 128, B], F32, tag="sm", name="l1ps")
            for j in range(L // 128):
                for e in range(2):
                    nc.tensor.matmul(l1ps[:, j, :],
                                     wl1r_sb[:, e, j * 128:(j + 1) * 128],
                                     cT_sb[:, :, e],
                                     start=(e == 0), stop=(e == 1))
            l1p_sb = rowC.tile([128, L // 128, B], F32, tag="l1p_sb")
            nc.vector.tensor_copy(l1p_sb[:], l1ps[:])
            l1p_ap = bass.AP(tensor=l1p_in.ap().tensor, offset=0,
                             ap=[[B, 128], [128 * B, L // 128], [1, B]])
            nc.sync.dma_start(out=l1p_ap, in_=l1p_sb[:])
            nc.gpsimd.collective_compute(
                "AllReduce", OP.add,
                ins=[l1p_in.ap()], outs=[l1p_out.ap()], replica_groups=RG)
            arT_sb = rowC.tile([128, L // 128, B], F32, tag="arT_sb")
            nc.sync.dma_start(
                out=arT_sb[:],
                in_=bass.AP(tensor=l1p_out.ap().tensor, offset=0,
                            ap=[[B, 128], [128 * B, L // 128], [1, B]]))
            l1b_sb = rowC.tile([128, L // 128, B], F32, tag="l1b_sb")
            nc.vector.tensor_add(l1b_sb[:], arT_sb[:], bl1t_sb[:])
            c1fT = rowC.tile([128, L // 128, B], BF, tag="c1fT")
            nc.vector.tensor_scalar_max(c1fT[:], l1b_sb[:], 0.0)
            ops = psSm.tile([B, OUT], F32, tag="sm", name="finps")
            for j in range(L // 128):
                nc.tensor.matmul(ops[:], c1fT[:, j, :], wl2_sb[:, j, :],
                                 start=(j == 0), stop=(j == L // 128 - 1))
            out_f = rowC.tile([B, OUT], F32, tag="out_f")
            nc.vector.tensor_add(out_f[:], ops[:], bl2_bc[:])
            nc.sync.dma_start(out=out.ap(), in_=out_f[:])

    nc.compile()
    return nc


def _to_bf16(a):
    return np.asarray(a, dtype=np.float32).astype(ml_dtypes.bfloat16)


def _to_f8(a):
    return np.asarray(a, dtype=np.float32).astype(ml_dtypes.float8_e4m3)


def kernel(**inputs):
    from concourse.bass_utils import run_bass_kernel_spmd

    gamma_np0 = np.asarray(inputs["gamma"], dtype=np.float32)
    beta_np0 = np.asarray(inputs["beta"], dtype=np.float32)
    trivial_gb = bool(np.all(gamma_np0 == 1.0) and np.all(beta_np0 == 0.0))
    key = ("nc", trivial_gb)
    if key not in _CACHE:
        _CACHE[key] = _build_nc(trivial_gb=trivial_gb)
    nc = _CACHE[key]

    x = np.asarray(inputs["x"], dtype=np.float32).reshape(N, D)
    isq = 1.0 / math.sqrt(DH)
    gamma_np = np.asarray(inputs["gamma"], dtype=np.float32)
    beta_np = np.asarray(inputs["beta"], dtype=np.float32)
    wc1_np = np.asarray(inputs["wc1"], dtype=np.float32)
    bc1_np = np.asarray(inputs["bc1"], dtype=np.float32)
    # fold LN2's gamma/beta into the c1 projection (h2 feeds only this matmul)
    wc1_f = gamma_np[:, None] * wc1_np
    bc1_f = bc1_np + beta_np @ wc1_np

    xT_f8 = np.ascontiguousarray(_to_f8(x).T)
    shared = dict(
        xT=xT_f8,
        wff=_to_bf16(inputs["wff"]),
        bff=_to_bf16(inputs["bff"]),
        gamma=_to_bf16(gamma_np), beta=_to_bf16(beta_np),
        wc1=_to_bf16(wc1_f), bc1=bc1_f.astype(np.float32),
        wc2=_to_bf16(np.asarray(inputs["wc2"]).reshape(D)),
        bc2=np.asarray(inputs["bc2"], np.float32).reshape(1),
        bl2=np.asarray(inputs["bl2"], np.float32),
    )
    wl1_np = np.asarray(inputs["wl1"], np.float32)
    bl1_np = np.asarray(inputs["bl1"], np.float32)
    wl2_np = np.asarray(inputs["wl2"], np.float32)
    shared["bl1t4"] = np.ascontiguousarray(
        np.repeat(bl1_np[:, None], B, axis=1)).astype(np.float32)
    shared["wl2f"] = _to_bf16(wl2_np)
    # fp8 weights pre-scaled by SCL so they sit in e4m3's normal range;
    # the scale is undone by ESC in exp() and the SCL-valued ones-vector
    wq = np.asarray(inputs["wq"], np.float32) * (isq * SCL)
    bq = np.asarray(inputs["bq"], np.float32) * (isq * SCL)
    wk = np.asarray(inputs["wk"], np.float32) * SCL
    bk = np.asarray(inputs["bk"], np.float32) * SCL
    wv = np.asarray(inputs["wv"], np.float32) * SCL
    bv = np.asarray(inputs["bv"], np.float32) * SCL

    in_maps = []
    for i in range(NC):
        sl = slice(i * DH, (i + 1) * DH)
        wqkv_i = np.stack([wq[:, sl], wk[:, sl], wv[:, sl]])
        bqkv_i = np.stack([bq[sl], bk[sl], bv[sl]])
        # rows this core owns after the A2A: for each batch b, L-positions
        # [i*LPC, (i+1)*LPC) -> 8 row-tiles of 128 = (b, e) pairs
        xr = np.concatenate([
            x[b * L + i * LPC: b * L + (i + 1) * LPC, :] for b in range(B)
        ])  # [RPC, D] ordered (b, l-within-block)
        in_maps.append(dict(
            shared,
            wqkv=_to_f8(wqkv_i),
            bqkv=bqkv_i.astype(np.float32),
            xrows=_to_bf16(xr),
            wl1r=_to_bf16(wl1_np[i * LPC:(i + 1) * LPC, :]),
        ))

    res = run_bass_kernel_spmd(nc, in_maps, core_ids=list(range(NC)))
    return np.asarray(res.results[0]["out"], dtype=np.float32)
